# revision 19
# baseline (speedup 1.0000x reference)
"""Multi-head attention (B=2, S=2048, D=1024, H=16) on 8 Trainium2 cores.

Sharding: core c -> batch b = c // 4, head group g = c % 4 (4 heads each).
Each core computes its 4 heads end-to-end (QKV proj -> attention -> out-proj
partial) and returns a partial [S, D] output; the host sums the 4 partials
per batch and adds the output bias.

Key HW facts this version is built around (measured via microbenchmarks):
  - A concurrent ScalarE ACTIVATE writing f32 halves PE matmul streaming
    (227ns -> 427ns for N=512); with bf16 matmul operands AND bf16 exp
    output both engines run at full rate simultaneously.  So everything
    the PE touches is bf16, and exp evacuates PSUM->SBUF as bf16.
  - Alternating stationary shapes defeat LDWEIGHTS pull-ahead; batch
    same-shape matmuls (2-kt blocks: 4 score MMs, 2 exps, 4 PV MMs).
  - exp is ScalarE-only at 1 elem/cycle/lane: 16.8M exps/core ~= 120us.
    The whole kernel is scheduled to hide all other work under that.

Per-core dataflow:
  K,Q proj (transposed orientation): K^T/Q^T = W^T @ X^T, hd on partitions.
  V proj (natural orientation): V = X^T.T @ Wv, keys on partitions -- no
    PE transposes needed; a ones column is appended for softmax denominators.
  scores S^T[k, q] = (K^T chunk).T @ Q^T chunk per (kt, head)  -> PSUM
  P = exp(S^T / 8)  ScalarE, [128, 1024] 2-bank reads, bf16 out
  U[hd+1, q] += (V~[k, hd+1]).T @ P[k, q]  accumulated over kt
  normalize: DVE reciprocal of U's sum row, GpSimd partition_broadcast,
    DVE multiply -> O^T bf16
  out-proj: partial[q, D] = sum_hp (O^T[hp] chunk).T @ Wo rows, DMA'd out.
"""

import functools
import numpy as np
import ml_dtypes
from contextlib import ExitStack

import concourse.bass as bass
import concourse.mybir as mybir
import concourse.tile as tile
from concourse import bacc
from concourse.bass import ts, ds
from concourse.bass_utils import run_bass_kernel_spmd

F32 = mybir.dt.float32
BF16 = mybir.dt.bfloat16
FP8 = mybir.dt.float8e4
AF = mybir.ActivationFunctionType
BF = ml_dtypes.bfloat16
# exp(score/8 - 2): the e^-2 shift keeps fp8e4 P values well under the
# TRN e4m3 240/NaN cliff; it cancels in the softmax normalization.
EXP_BIAS = -2.0
VPAD = 68  # hd+1 padded so the kt-axis byte stride (HC*VPAD) is 16-aligned

B, S, D = 2, 2048, 1024
H_TOT, HD = 16, 64
HC = 4                 # heads per core
DC = HC * HD           # 256 columns of QKV proj per core
NCORES = 8
P = 128
NDT = D // P           # 8 d-model tiles
NKT = S // P           # 16 key tiles
CG = 512               # q chunk width
NCG = S // CG          # 4
SCALE = 1.0 / np.sqrt(HD)


DBG = False


def _body(ctx, tc, xq, xk, xv, wq, wk, wv, bq, bk, bv, wo, outp, dbg=None):
    nc = tc.nc

    singles = ctx.enter_context(tc.tile_pool(name="singles", bufs=1))
    wpool = ctx.enter_context(tc.tile_pool(name="wpool", bufs=3))
    xpool = ctx.enter_context(tc.tile_pool(name="xpool", bufs=3))
    ppool = ctx.enter_context(tc.tile_pool(name="ppool", bufs=6))
    npool = ctx.enter_context(tc.tile_pool(name="npool", bufs=2))
    opool = ctx.enter_context(tc.tile_pool(name="opool", bufs=3))
    psS = ctx.enter_context(tc.tile_pool(name="psS", bufs=2, space="PSUM"))
    psU = ctx.enter_context(tc.tile_pool(name="psU", bufs=2, space="PSUM"))
    psX = ctx.enter_context(tc.tile_pool(name="psX", bufs=2, space="PSUM"))

    # Persistent per-core tensors
    QT = [singles.tile([P, S], BF16, tag=f"qt{m}", name=f"qt{m}") for m in range(2)]
    # KT[hp]: natural K^T -- rows 0..63 hold head 2hp, rows 64..127 head
    # 2hp+1.  Scores run as two concurrent K=64 row-tiled matmuls
    # (tile_position (0,0) and (64,0) auto-derived from base partitions),
    # sharing one 512-col stream slot -> 2x score throughput vs K=128.
    KT = [singles.tile([P, S], BF16, tag=f"kt{m}", name=f"kt{m}") for m in range(2)]
    OT = [singles.tile([P, S], BF16, tag=f"ot{m}", name=f"ot{m}") for m in range(2)]
    Vt = singles.tile([P, NKT, HC, HD + 1], BF16, tag="vtile")   # [keys, kt, h, hd+1]
    wo_sb = singles.tile([P, 2, D], BF16, tag="wo")
    bvt = singles.tile([P, DC], F32, tag="bvt")   # V bias broadcast across partitions
    bvrow = singles.tile([1, DC], F32, tag="bvrow")

    # ---- preamble: act-table warm-up + V bias broadcast (also loads gpsimd lib)
    ebias = singles.tile([P, 1], F32, tag="ebias")
    nc.vector.memset(ebias, EXP_BIAS)
    warm_sb = opool.tile([P, D], F32, tag="ob")
    nc.vector.memset(warm_sb[:, 0:4], 0.0)
    nc.scalar.activation(out=warm_sb[:, 2:4], in_=warm_sb[:, 0:2], func=AF.Exp, scale=0.125)
    nc.vector.memset(Vt[:, :, :, HD:HD + 1], 1.0)
    # junk matmuls while the first weight/X DMAs land: keeps the HAM clock
    # gate warm so the first real matmuls run at 2.4 GHz
    warm_ps = psX.tile([P, CG], F32, tag="aux", name="warmps")
    wjunk = singles.tile([P, P], BF16, tag="wjunk")
    nc.vector.memset(wjunk, 0.5)
    for r in range(14):
        nc.tensor.matmul(warm_ps[:, ts(r % 2, 64)], lhsT=wjunk, rhs=wjunk[:, 0:64],
                         start=True, stop=True)

    # ---- projection emitters ----
    # The preamble runs only what the FIRST exp needs (K keys 0..511 and
    # Q chunk 0, m=0 halves, plus a short V head start); everything else --
    # remaining K chunks, V s-tiles, Q m=1 and later chunks -- is emitted
    # as deadline-ordered foreign groups popped under the exp stream, so
    # ScalarE starts ~10us in instead of ~54us.
    def slab_dma(xslab, x_dram, cg, split=False):
        if split:
            # per-dt sub-DMAs: fine-grained deps so the first matmul can
            # start ~1us in (used only for critical-path slabs)
            for dt in range(NDT):
                nc.sync.dma_start(out=xslab[:, dt, :], in_=x_dram[ds(dt * P, P), ts(cg, CG)])
        else:
            nc.sync.dma_start(
                out=xslab, in_=x_dram[:, ts(cg, CG)].rearrange("(t p) q -> p t q", p=P)
            )

    def proj_half_emitters(w_sb, b_sb, xslab, m, DEST, cg, nm):
        # one m-half of a K/Q projection chunk: 8 accumulating matmuls
        # into a psX bank, then a bias-add into DEST[m][:, cg chunk]
        box = {}

        def alloc(box=box):
            box["aux"] = psX.tile([P, CG], F32, tag="aux", name=f"pj{nm}")

        def mm(dt, box=box):
            nc.tensor.matmul(
                box["aux"],
                lhsT=w_sb[:, dt, ts(m, P)],
                rhs=xslab[:, dt, :],
                start=(dt == 0),
                stop=(dt == NDT - 1),
            )

        def fin(box=box):
            nc.vector.tensor_scalar_add(
                out=DEST[m][:, ts(cg, CG)], in0=box["aux"],
                scalar1=b_sb[:, m:m + 1],
            )

        return [alloc] + [functools.partial(mm, dt) for dt in range(NDT)] + [fin]

    def vproj_emitters(st, xslab):
        # one V s-tile: V[s128, 256] = sum_dt XvT[:, dt, s].T @ Wv[dt]
        sti = st % (CG // P)
        box = {}

        def alloc(box=box):
            box["aux"] = psX.tile([P, CG], F32, tag="aux", name=f"vp{st}")

        def mm(dt, box=box):
            nc.tensor.matmul(
                box["aux"][:, 0:DC],
                lhsT=xslab[:, dt, ts(sti, P)],
                rhs=wv_sb[:, dt, :],
                start=(dt == 0),
                stop=(dt == NDT - 1),
            )

        def fin(box=box):
            vsb = npool.tile([P, DC], BF16, tag="vsb")
            nc.vector.tensor_add(out=vsb, in0=box["aux"][:, 0:DC], in1=bvt)
            nc.vector.tensor_copy(
                out=Vt[:, st, :, 0:HD],
                in_=vsb.rearrange("p (h e) -> p h e", e=HD),
            )

        return [alloc] + [functools.partial(mm, dt) for dt in range(NDT)] + [fin]

    # weight/bias/slab DMAs, most-urgent first
    wk_sb = wpool.tile([P, NDT, DC], BF16, tag="w", name="wk")
    for dt in range(NDT):
        nc.sync.dma_start(out=wk_sb[:, dt, :], in_=wk[ds(dt * P, P), :])
    bk_sb = wpool.tile([P, 2], F32, tag="b", name="bk")
    nc.sync.dma_start(out=bk_sb, in_=bk.rearrange("(m p) -> p m", p=P))
    xk_slabs = [xpool.tile([P, NDT, CG], BF16, tag="xk", bufs=2, name=f"xk{c}")
                for c in range(NCG)]
    slab_dma(xk_slabs[0], xk, 0, split=True)
    wq_sb = wpool.tile([P, NDT, DC], BF16, tag="w", name="wq")
    nc.sync.dma_start(out=wq_sb, in_=wq.rearrange("(t p) c -> p t c", p=P))
    bq_sb = wpool.tile([P, 2], F32, tag="b", name="bq")
    nc.sync.dma_start(out=bq_sb, in_=bq.rearrange("(m p) -> p m", p=P))
    xq_slabs = {0: xpool.tile([P, NDT, CG], BF16, tag="xq", bufs=2, name="xq0")}
    slab_dma(xq_slabs[0], xq, 0, split=True)
    wv_sb = wpool.tile([P, NDT, DC], BF16, tag="w", name="wv")
    nc.sync.dma_start(out=wv_sb, in_=wv.rearrange("(t p) c -> p t c", p=P))
    xv_slabs = [xpool.tile([P, NDT, CG], BF16, tag="xv", bufs=2, name=f"xv{c}")
                for c in range(NCG)]
    slab_dma(xv_slabs[0], xv, 0)
    nc.sync.dma_start(out=bvrow, in_=bv.rearrange("(o c) -> o c", o=1))
    nc.gpsimd.partition_broadcast(bvt, bvrow)
    slab_dma(xv_slabs[1], xv, 1)
    slab_dma(xk_slabs[1], xk, 1)
    nc.sync.dma_start(out=wo_sb, in_=wo.rearrange("(k p) d -> p k d", p=P))
    slab_dma(xv_slabs[2], xv, 2)
    slab_dma(xk_slabs[2], xk, 2)
    slab_dma(xv_slabs[3], xv, 3)
    slab_dma(xk_slabs[3], xk, 3)

    # inline preamble: just K chunk0 m0 and Q chunk0 m0
    for em in proj_half_emitters(wk_sb, bk_sb, xk_slabs[0], 0, KT, 0, "k00"):
        em()
    for em in proj_half_emitters(wq_sb, bq_sb, xq_slabs[0], 0, QT, 0, "q00"):
        em()

    # deadline-ordered foreign work for cg0 (V st popped at PV site st,
    # K chunk c m0 by scores site 4c, m1 halves by hp1; Q cg1 last)
    def kproj_ems(c, m):
        return proj_half_emitters(wk_sb, bk_sb, xk_slabs[c], m, KT, c, f"k{c}{m}")

    # need[label] = foreign-list index that must be popped before the
    # consumer of `label` is emitted (Tile semantics are emission-order:
    # a producer emitted after its consumer reads as uninitialized)
    preforeign = []
    need = {}

    def addpre(ems, label=None):
        preforeign.extend(ems)
        if label is not None:
            need[label] = len(preforeign)

    for st in (0, 1, 2, 3):
        addpre(vproj_emitters(st, xv_slabs[0]), ("V", st))
    addpre(kproj_ems(0, 1), ("K", 0, 1))
    addpre(proj_half_emitters(wq_sb, bq_sb, xq_slabs[0], 1, QT, 0, "q01"),
           ("Q", 0, 1))
    addpre(kproj_ems(1, 0), ("K", 1, 0))
    for st in (4, 5):
        addpre(vproj_emitters(st, xv_slabs[1]), ("V", st))
    addpre(kproj_ems(1, 1), ("K", 1, 1))
    for st in (6, 7):
        addpre(vproj_emitters(st, xv_slabs[1]), ("V", st))
    addpre(kproj_ems(2, 0), ("K", 2, 0))
    for st in (8, 9):
        addpre(vproj_emitters(st, xv_slabs[2]), ("V", st))
    addpre(kproj_ems(2, 1), ("K", 2, 1))
    for st in (10, 11):
        addpre(vproj_emitters(st, xv_slabs[2]), ("V", st))
    addpre(kproj_ems(3, 0), ("K", 3, 0))
    for st in (12, 13):
        addpre(vproj_emitters(st, xv_slabs[3]), ("V", st))
    addpre(kproj_ems(3, 1), ("K", 3, 1))
    for st in (14, 15):
        addpre(vproj_emitters(st, xv_slabs[3]), ("V", st))

    # ---- attention main loop ----
    # Scores run in 64x128 row-tile mode: per kt, head-even on PE rows
    # 0..63 (tile (0,0)) and head-odd on rows 64..127 (tile (64,0)) execute
    # CONCURRENTLY, sharing one 512-col stream slot -- 2x score throughput.
    # The two tiles write different PSUM banks of one [P,2,CG] tile, and
    # each bank is written by exactly one PE tile (reorder-proof: concurrent
    # tiles on one bank are a fatal HW collision).
    # Everything else (PV, proj, out-proj) stays in full 128x128 mode.
    # A tile-mode switch costs ~130ns and un-pairs the switching MM, so
    # scores are emitted in 2-kt clusters: one 64-mode window (4 paired
    # MMs) then one 128-mode window (PVs + foreign pops) per cluster.

    def qproj_emitters(cg):
        xslab = xpool.tile([P, NDT, CG], BF16, tag="xq", bufs=2, name=f"xq{cg}")
        xq_slabs[cg] = xslab
        slab_dma(xslab, xq, cg)
        return (
            proj_half_emitters(wq_sb, bq_sb, xslab, 0, QT, cg, f"q{cg}0")
            + proj_half_emitters(wq_sb, bq_sb, xslab, 1, QT, cg, f"q{cg}1")
        )

    def outproj_emitters(cg, pool):
        # per (qt, 512-col chunk): 2 accumulating matmuls -> evac -> DMA.
        ems = []
        for qt in range(CG // P):
            q0 = cg * (CG // P) + qt
            for c2 in range(2):
                box = {}

                def alloc(box=box, qt=qt, c2=c2):
                    if pool is psS:
                        t = psS.tile([P, 2, CG], F32, tag="s", name=f"opaux{cg}_{qt}_{c2}")
                        box["aux"] = t[:, 0, :]
                    else:
                        box["aux"] = psX.tile([P, CG], F32, tag="aux", name=f"opaux{cg}_{qt}_{c2}")

                def mm(k2, box=box, q0=q0, c2=c2):
                    nc.tensor.matmul(
                        box["aux"],
                        lhsT=OT[k2][:, ts(q0, P)],
                        rhs=wo_sb[:, k2, ts(c2, CG)],
                        start=(k2 == 0),
                        stop=(k2 == 1),
                    )

                def fin(box=box, q0=q0, c2=c2):
                    ob = opool.tile([P, CG], F32, tag="ob", name=f"ob{q0}_{c2}")
                    nc.vector.tensor_copy(out=ob, in_=box["aux"])
                    nc.sync.dma_start(out=outp[ts(q0, P), ts(c2, CG)], in_=ob)

                ems.append(alloc)
                for k2 in range(2):
                    ems.append(functools.partial(mm, k2))
                ems.append(fin)
        return ems

    NCL = NKT // 2  # 2-kt score clusters per (cg, hp)

    for cg in range(NCG):
        foreign = []
        if cg == 0:
            foreign += preforeign
        if cg + 1 < NCG:
            foreign += qproj_emitters(cg + 1)
        if cg >= 1:
            foreign += outproj_emitters(cg - 1, psX)
        npops = 2 * (NCL - 1)
        npop = (len(foreign) + npops - 1) // npops if foreign else 0
        popcnt = [0]

        def pops(n):
            for _ in range(n):
                if foreign:
                    foreign.pop(0)()
                    popcnt[0] += 1

        def ensure(label):
            # emission-order deadline: pop foreign until `label`'s
            # producer group has been emitted
            tgt = need.get(label, 0) if cg == 0 else 0
            if tgt > popcnt[0]:
                pops(tgt - popcnt[0])

        for hp in range(2):
            U = [psU.tile([HD + 1, CG], F32, tag="u", name=f"u{i}") for i in range(2)]

            def pv(kt, p2, hp=hp, U=U):
                # PVs run one cluster behind their exps so the PV weight
                # load isn't gated on the exp semaphore (LDWEIGHTS pulls
                # ahead); full-mode M=65 (ones column -> denominator row).
                for i in range(2):
                    nc.tensor.matmul(
                        U[i],
                        lhsT=Vt[:, kt, 2 * hp + i, 0:HD + 1],
                        rhs=p2[:, i, :],
                        start=(kt == 0),
                        stop=(kt == NKT - 1),
                    )

            pending = []
            for cl in range(NCL):
                p2s = []
                for kt in (2 * cl, 2 * cl + 1):
                    ensure(("K", kt // 4, hp))
                    ensure(("Q", 0, hp))
                    s2 = psS.tile([P, 2, CG], F32, tag="s", name="s")
                    for i in range(2):
                        sl = slice(64 * i, 64 * i + 64)
                        nc.tensor.matmul(
                            s2[:, i, :],
                            lhsT=KT[hp][sl, ts(kt, P)],
                            rhs=QT[hp][sl, ts(cg, CG)],
                            start=True,
                            stop=True,
                        )
                    p2 = ppool.tile([P, 2, CG], BF16, tag="p", name="p")
                    nc.scalar.activation(
                        out=p2, in_=s2, func=AF.Exp,
                        scale=float(SCALE), bias=ebias,
                    )
                    p2s.append((kt, p2))
                for kt_, pp in pending:
                    ensure(("V", kt_))
                    pv(kt_, pp)
                pending = p2s
                if cl >= 1:
                    pops(npop)
            for kt_, pp in pending:
                ensure(("V", kt_))
                pv(kt_, pp)

            # softmax normalize.  DVE part (U evac + denominator reciprocal)
            # runs now -- this frees the U PSUM ring early.  The PE part
            # (K=1 broadcast matmul) and the final multiply are deferred into
            # the next stage so they never head-of-line block the PE queue.
            if DBG and cg == 0 and hp == 0:
                usb_d = opool.tile([HD + 1, CG], F32, tag="ob")
                nc.vector.tensor_copy(out=usb_d, in_=U[0])
                nc.sync.dma_start(out=dbg["u"], in_=usb_d)
            usbs, recs, bcs = [], [], []
            for i in range(2):
                # reciprocal_approx_fast ignores partition offsets on HW --
                # copy the denominator row to partition 0 first (tensor_copy
                # handles offsets correctly).  NOTE: must stay on DVE -- a
                # ScalarE copy here head-of-line blocks the next stage's exps.
                drow = npool.tile([1, CG], F32, tag="drow", name=f"drow{i}")
                nc.vector.tensor_copy(out=drow, in_=U[i][HD:HD + 1, :])
                rec = npool.tile([1, CG], F32, tag="rec", name=f"rec{i}")
                nc.vector.reciprocal_approx_fast(out=rec, in_=drow)
                usb = npool.tile([HD, CG], F32, tag="usb", name=f"usb{i}")
                nc.vector.tensor_copy(out=usb, in_=U[i][0:HD, :])
                usbs.append(usb)
                recs.append(rec)
            for i in range(2):
                bc = npool.tile([64, CG], F32, tag="bc", name=f"bc{i}")
                nc.gpsimd.partition_broadcast(bc, recs[i])
                bcs.append(bc)
            for i in range(2):
                nc.vector.tensor_mul(
                    out=OT[hp][64 * i:64 * i + 64, ts(cg, CG)],
                    in0=usbs[i],
                    in1=bcs[i],
                )

        for em in foreign:   # drain any leftovers before the next cg
            em()

    # tail: a few full-mode fillers gated on the last norm outputs keep the
    # HAM clock warm through the norm-chain gap, then the last chunk's
    # out-proj on the now-free psS banks
    warm2 = psX.tile([P, CG], F32, tag="aux", name="warmps2")
    for r in range(4):
        nc.tensor.matmul(
            warm2[:, 0:64],
            lhsT=OT[1][:, ds(S - P, P)],
            rhs=OT[0][:, ds(S - 64, 64)],
            start=True, stop=True)
    for em in outproj_emitters(NCG - 1, psS):
        em()

    if DBG:
        for m in range(2):
            t = opool.tile([P, S], F32, tag="dbg", bufs=1)
            nc.vector.tensor_copy(out=t, in_=QT[m])
            nc.sync.dma_start(out=dbg[f"qt{m}"], in_=t)
            t2 = opool.tile([P, S], F32, tag="dbg", bufs=1)
            nc.vector.tensor_copy(out=t2, in_=KT[m])
            nc.sync.dma_start(out=dbg[f"kt{m}"], in_=t2)
            t3 = opool.tile([P, S], F32, tag="dbg", bufs=1)
            nc.vector.tensor_copy(out=t3, in_=OT[m])
            nc.sync.dma_start(out=dbg[f"ot{m}"], in_=t3)
        tv = opool.tile([P, NKT * HC * (HD + 1)], F32, tag="dbg", bufs=1)
        nc.vector.tensor_copy(out=tv, in_=Vt.rearrange("p a b c -> p (a b c)"))
        nc.sync.dma_start(out=dbg["vt"], in_=tv)


def build_nc():
    nc = bacc.Bacc("TRN2", target_bir_lowering=False, debug=False)
    aps = {}
    for name, shape, dt_ in (
        ("xq", [D, S], BF16),
        ("xk", [D, S], BF16),
        ("xv", [D, S], BF16),
        ("wq", [D, DC], BF16),
        ("wk", [D, DC], BF16),
        ("wv", [D, DC], BF16),
        ("bq", [DC], F32),
        ("bk", [DC], F32),
        ("bv", [DC], F32),
        ("wo", [DC, D], BF16),
    ):
        aps[name] = nc.dram_tensor(name, shape, dt_, kind="ExternalInput").ap()
    aps["outp"] = nc.dram_tensor("out_partial", [S, D], F32, kind="ExternalOutput").ap()
    dbg = None
    if DBG:
        dbg = {}
        for m in range(2):
            dbg[f"qt{m}"] = nc.dram_tensor(f"dbg_qt{m}", [P, S], F32, kind="ExternalOutput").ap()
            dbg[f"kt{m}"] = nc.dram_tensor(f"dbg_kt{m}", [P, S], F32, kind="ExternalOutput").ap()
            dbg[f"ot{m}"] = nc.dram_tensor(f"dbg_ot{m}", [P, S], F32, kind="ExternalOutput").ap()
        dbg["vt"] = nc.dram_tensor("dbg_vt", [P, NKT * HC * (HD + 1)], F32, kind="ExternalOutput").ap()
        dbg["u"] = nc.dram_tensor("dbg_u", [HD + 1, CG], F32, kind="ExternalOutput").ap()
        dbg["rec"] = nc.dram_tensor("dbg_rec", [1, CG], F32, kind="ExternalOutput").ap()
        dbg["bc"] = nc.dram_tensor("dbg_bc", [64, CG], F32, kind="ExternalOutput").ap()

    with tile.TileContext(nc) as tc:
        with ExitStack() as ctx:
            _body(
                ctx,
                tc,
                aps["xq"], aps["xk"], aps["xv"],
                aps["wq"], aps["wk"], aps["wv"],
                aps["bq"], aps["bk"], aps["bv"],
                aps["wo"], aps["outp"], dbg,
            )
    nc.compile()
    return nc


def make_in_maps(inputs):
    q = np.asarray(inputs["query"], dtype=np.float32)
    k = np.asarray(inputs.get("key_", inputs.get("key")), dtype=np.float32)
    v = np.asarray(inputs["value"], dtype=np.float32)
    Wq = np.asarray(inputs["Wq"], dtype=np.float32)
    Wk = np.asarray(inputs["Wk"], dtype=np.float32)
    Wv = np.asarray(inputs["Wv"], dtype=np.float32)
    bq = np.asarray(inputs["bq"], dtype=np.float32)
    bk = np.asarray(inputs["bk"], dtype=np.float32)
    bv = np.asarray(inputs["bv"], dtype=np.float32)
    Wo = np.asarray(inputs["Wo"], dtype=np.float32)

    # one host transpose+cast per batch, shared by the 4 cores of that batch
    qT = [np.ascontiguousarray(q[b].T).astype(BF) for b in range(B)]
    kT = [np.ascontiguousarray(k[b].T).astype(BF) for b in range(B)]
    vT = [np.ascontiguousarray(v[b].T).astype(BF) for b in range(B)]

    in_maps = []
    for c in range(NCORES):
        b, g = divmod(c, 4)
        cs = slice(DC * g, DC * (g + 1))
        in_maps.append(
            {
                "xq": qT[b],
                "xk": kT[b],
                "xv": vT[b],
                "wq": np.ascontiguousarray(Wq[:, cs]).astype(BF),
                "wk": np.ascontiguousarray(Wk[:, cs]).astype(BF),
                "wv": np.ascontiguousarray(Wv[:, cs]).astype(BF),
                "bq": np.ascontiguousarray(bq[cs]),
                "bk": np.ascontiguousarray(bk[cs]),
                "bv": np.ascontiguousarray(bv[cs]),
                "wo": np.ascontiguousarray(Wo[cs, :]).astype(BF),
            }
        )
    return in_maps


_NC_CACHE = {}


def get_nc():
    if "nc" not in _NC_CACHE:
        _NC_CACHE["nc"] = build_nc()
    return _NC_CACHE["nc"]


def kernel(**inputs):
    nc = get_nc()
    in_maps = make_in_maps(inputs)
    res = run_bass_kernel_spmd(nc, in_maps, list(range(NCORES))).results
    bo = np.asarray(inputs["bo"], dtype=np.float32)
    out = np.empty((B, S, D), dtype=np.float32)
    for b in range(B):
        acc = res[4 * b + 0]["out_partial"].astype(np.float32)
        for g in range(1, 4):
            acc = acc + res[4 * b + g]["out_partial"]
        out[b] = acc + bo[None, :]
    return out

